# revision 1
# baseline (speedup 1.0000x reference)
"""Trainium2 Bass kernel for nn_AttentionBlock (B=16, C=512, H=W=64, 8 heads).

Channel-attention block: GroupNorm(8 groups) -> 1x1 qkv -> scores over
channel dims (contract spatial N=4096) -> softmax -> att @ v -> 1x1 out
projection -> residual.

Sharding: data-parallel over batch. 16 batches / 8 cores = 2 per core.
No collectives. Each core runs the identical program on its 2 batches.

Layouts on device (per batch):
  x     [C, N] fp32, 4 channel-chunk tiles of [128, 4096]
  h     (groupnorm output) same layout, bf16
  q,k   [N, 2C] orientation (spatial on partitions), bf16, transient tiles
  v     [C, N] bf16, resident
  scores 8 heads of [64, 64] packed into two [128, 128] psum tiles
  hv    [C, N] bf16 via paired-head matmuls
  out   = w_out @ hv + (w_out @ (att @ b_v) + b_out) + x   (residual)

All matmuls bf16 inputs with fp32 psum accumulation; groupnorm stats,
softmax, and the residual path are fp32.
"""

import numpy as np
import ml_dtypes

import concourse.bacc as bacc
import concourse.tile as tile
from concourse import mybir
from concourse.bass_utils import run_bass_kernel_spmd
from concourse.masks import make_identity

BF = mybir.dt.bfloat16
F32 = mybir.dt.float32
AX = mybir.AxisListType
OP = mybir.AluOpType
AF = mybir.ActivationFunctionType

C = 512
NH = 8
D = 64  # head dim
G = 8   # groupnorm groups
CK = C // 128  # 4 channel chunks
EPS = 1e-5
N_CORES = 8

# attT slot coords inside a [128,128] attT tile, per chunk parity.
# chunk ck holds heads (2ck, 2ck+1); tile tt = ck // 2.
# even head lhsT lives at partitions 0:64, odd head at partitions 64:128.
_EVEN_SLOT = {0: (0, 0), 1: (0, 64)}   # ck%2 -> (prow, colstart)
_ODD_SLOT = {0: (64, 64), 1: (64, 0)}
# scores placement: local head l (0..3) -> (prow, colstart) in scores tile
_SCORE_SLOT = {0: (0, 0), 1: (64, 64), 2: (64, 0), 3: (0, 64)}


def build_program(B=2, N=4096, debug=False):
    SP = N // 128   # spatial chunks for qk/scores
    NT = N // 512   # 512-col tiles
    SUB = N // 512  # bn_stats subgroups (free dim <= 512)
    scale = float(1.0 / np.sqrt(D))

    nc = bacc.Bacc("TRN2", target_bir_lowering=False, debug=debug,
                   num_devices=N_CORES)

    x_d = nc.dram_tensor("x", [B, C, N], F32, kind="ExternalInput")
    wqk_d = nc.dram_tensor("wqkT", [C, 2 * C], BF, kind="ExternalInput")
    wv_d = nc.dram_tensor("wvT", [C, C], BF, kind="ExternalInput")
    wo_d = nc.dram_tensor("woT", [C, C], BF, kind="ExternalInput")
    bqk_d = nc.dram_tensor("bqk", [1, 2 * C], BF, kind="ExternalInput")
    bv_d = nc.dram_tensor("bv", [C, 1], BF, kind="ExternalInput")
    bo_d = nc.dram_tensor("bo", [C, 1], F32, kind="ExternalInput")
    gam_d = nc.dram_tensor("gamma", [C, 1], F32, kind="ExternalInput")
    bet_d = nc.dram_tensor("beta", [C, 1], F32, kind="ExternalInput")
    indf_d = nc.dram_tensor("indf", [C, G], F32, kind="ExternalInput")
    indb_d = nc.dram_tensor("indb", [G, C], F32, kind="ExternalInput")
    out_d = nc.dram_tensor("out", [B, C, N], F32, kind="ExternalOutput")

    with tile.TileContext(nc) as tc:
        import contextlib
        ctx = contextlib.ExitStack()
        with ctx:
            persist = ctx.enter_context(tc.tile_pool(name="persist", bufs=1))
            big = ctx.enter_context(tc.tile_pool(name="big", bufs=1))
            mid = ctx.enter_context(tc.tile_pool(name="mid", bufs=3))
            small = ctx.enter_context(tc.tile_pool(name="small", bufs=1))
            ps_qk = ctx.enter_context(
                tc.tile_pool(name="ps_qk", bufs=3, space="PSUM"))
            ps_sc = ctx.enter_context(
                tc.tile_pool(name="ps_sc", bufs=1, space="PSUM"))
            ps_big = ctx.enter_context(
                tc.tile_pool(name="ps_big", bufs=2, space="PSUM"))

            # ---- persistent: weights / constants ----
            wqk = []
            wv = []
            wo = []
            bv_sb = []
            bo_sb = []
            gam = []
            bet = []
            for k in range(CK):
                t = persist.tile([128, 2 * C], BF, tag=f"wqk{k}")
                nc.gpsimd.dma_start(out=t, in_=wqk_d.ap()[k * 128:(k + 1) * 128, :])
                wqk.append(t)
                t = persist.tile([128, C], BF, tag=f"wv{k}")
                nc.gpsimd.dma_start(out=t, in_=wv_d.ap()[k * 128:(k + 1) * 128, :])
                wv.append(t)
                t = persist.tile([128, C], BF, tag=f"wo{k}")
                nc.gpsimd.dma_start(out=t, in_=wo_d.ap()[k * 128:(k + 1) * 128, :])
                wo.append(t)
                t = persist.tile([128, 1], BF, tag=f"bv{k}")
                nc.gpsimd.dma_start(out=t, in_=bv_d.ap()[k * 128:(k + 1) * 128, :])
                bv_sb.append(t)
                t = persist.tile([128, 1], F32, tag=f"bo{k}")
                nc.gpsimd.dma_start(out=t, in_=bo_d.ap()[k * 128:(k + 1) * 128, :])
                bo_sb.append(t)
                t = persist.tile([128, 1], F32, tag=f"gam{k}")
                nc.gpsimd.dma_start(out=t, in_=gam_d.ap()[k * 128:(k + 1) * 128, :])
                gam.append(t)
                t = persist.tile([128, 1], F32, tag=f"bet{k}")
                nc.gpsimd.dma_start(out=t, in_=bet_d.ap()[k * 128:(k + 1) * 128, :])
                bet.append(t)
            # q/k bias replicated across all 128 partitions (spatial rows)
            import concourse.bass as bass
            bqk_rep = persist.tile([128, 2 * C], BF, tag="bqk_rep")
            _bqk_ap = bqk_d.ap()
            nc.gpsimd.dma_start(
                out=bqk_rep,
                in_=bass.AP(tensor=_bqk_ap.tensor, offset=_bqk_ap.offset,
                            ap=[[0, 128], [1, 2 * C]]))

            zero1 = persist.tile([1, 128], BF, tag="zero1")
            nc.gpsimd.memset(zero1, 0.0)
            zrhs256 = persist.tile([1, 256], BF, tag="zrhs256")
            nc.gpsimd.memset(zrhs256, 0.0)
            ident = persist.tile([128, 128], BF, tag="ident")
            make_identity(nc, ident)
            eps_t = persist.tile([128, 1], F32, tag="eps")
            nc.gpsimd.memset(eps_t, EPS)
            # group indicator matrices (groupnorm cross-partition reduce)
            indf = []
            for k in range(CK):
                t = persist.tile([128, G], F32, tag=f"indf{k}")
                nc.gpsimd.dma_start(
                    out=t, in_=indf_d.ap()[k * 128:(k + 1) * 128, :])
                indf.append(t)
            indb = persist.tile([G, C], F32, tag="indb")
            nc.gpsimd.dma_start(out=indb, in_=indb_d.ap())

            # ---- per-batch phases (emitted software-pipelined below) ----
            def phase_norm(b):
                # x load (split DMAs so bn_stats can start on early columns)
                xs = []
                for k in range(CK):
                    t = big.tile([128, N], F32, tag=f"x{k}")
                    xq = min(1024, N)
                    for q4 in range(0, N, xq):
                        nc.sync.dma_start(
                            out=t[:, q4:q4 + xq],
                            in_=x_d.ap()[b, k * 128:(k + 1) * 128,
                                         q4:q4 + xq])
                    xs.append(t)

                # groupnorm stats: per-partition mean/var via bn_stats
                mvs = []
                for k in range(CK):
                    st = small.tile([128, SUB, 6], F32, tag=f"st{k}")
                    for j in range(SUB):
                        nc.vector.bn_stats(
                            out=st[:, j, :], in_=xs[k][:, j * 512:(j + 1) * 512])
                    mv = small.tile([128, 2], F32, tag=f"mv{k}")
                    nc.vector.bn_aggr(out=mv, in_=st)
                    mvs.append(mv)
                # rhs2: col0 = mean_p, col1 = mean_p^2 + var_p = E[x^2]_p
                rhs2s = []
                for k in range(CK):
                    r2 = small.tile([128, 2], F32, tag=f"r2{k}")
                    nc.gpsimd.tensor_copy(out=r2[:, 0:1], in_=mvs[k][:, 0:1])
                    nc.vector.scalar_tensor_tensor(
                        out=r2[:, 1:2], in0=mvs[k][:, 0:1],
                        scalar=mvs[k][:, 0:1], in1=mvs[k][:, 1:2],
                        op0=OP.mult, op1=OP.add)
                    rhs2s.append(r2)
                # cross-partition reduce to per-group stats [8, 2]
                pg = ps_big.tile([G, 2], F32, tag="pbig")
                for k in range(CK):
                    nc.tensor.matmul(pg, indf[k], rhs2s[k],
                                     start=(k == 0), stop=(k == CK - 1))
                sg = small.tile([G, 2], F32, tag="sg")
                nc.vector.tensor_copy(out=sg, in_=pg)
                t2 = small.tile([G, 1], F32, tag="t2")
                nc.vector.tensor_mul(out=t2, in0=sg[:, 0:1], in1=sg[:, 0:1])
                vs = small.tile([G, 1], F32, tag="vs")
                nc.vector.tensor_sub(out=vs, in0=sg[:, 1:2], in1=t2)
                # rstd = exp(-0.5 * ln(var + eps)); Ln/Exp share a table set
                lnv = small.tile([G, 1], F32, tag="lnv")
                nc.scalar.activation(out=lnv, in_=vs, func=AF.Ln,
                                     bias=eps_t[0:G, :], scale=1.0)
                rstd = small.tile([G, 1], F32, tag="rstd")
                nc.scalar.activation(out=rstd, in_=lnv, func=AF.Exp, scale=-0.5)
                bcr = small.tile([G, 2], F32, tag="bcr")
                nc.gpsimd.tensor_copy(out=bcr[:, 0:1], in_=sg[:, 0:1])
                nc.gpsimd.tensor_copy(out=bcr[:, 1:2], in_=rstd)
                # broadcast group stats back to channels; affine coeffs
                scs = []
                nbs = []
                for k in range(CK):
                    pbc = ps_big.tile([128, 2], F32, tag="pbig")
                    nc.tensor.matmul(pbc, indb[:, k * 128:(k + 1) * 128], bcr,
                                     start=True, stop=True)
                    sc = small.tile([128, 1], F32, tag=f"sc{k}")
                    nc.vector.tensor_mul(out=sc, in0=pbc[:, 1:2], in1=gam[k])
                    t4 = small.tile([128, 1], F32, tag=f"t4{k}")
                    nc.vector.tensor_scalar_mul(out=t4, in0=pbc[:, 0:1],
                                                scalar1=sc)
                    nb = small.tile([128, 1], F32, tag=f"nb{k}")
                    nc.vector.tensor_sub(out=nb, in0=bet[k], in1=t4)
                    scs.append(sc)
                    nbs.append(nb)

                # normalize: h = x * scale_c + bias_c  (bf16).
                # Column-major loop order: the first qk matmul needs the
                # first 128 columns of ALL FOUR chunks, so producing columns
                # across chunks first lets the consumer start ~9us earlier
                # than chunk-major order would.
                hs = []
                for k in range(CK):
                    hs.append(big.tile([128, N], BF, tag=f"h{k}",
                                       name=f"h{k}"))
                for t in range(NT):
                    sl = slice(t * 512, (t + 1) * 512)
                    for k in range(CK):
                        nc.vector.tensor_scalar(
                            out=hs[k][:, sl], in0=xs[k][:, sl],
                            scalar1=scs[k], scalar2=nbs[k],
                            op0=OP.mult, op1=OP.add)
                return hs

            def phase_qkv_setup(b):
                # scores accumulators: both packed tiles share one psum bank
                Tsc = ps_sc.tile([128, 256], F32, tag="sc01")
                T0 = Tsc[:, 0:128]
                T1 = Tsc[:, 128:256]
                # one full-width zeroing matmul: marks the bank's pending-zero
                # bits and writes 0 everywhere; every scores matmul overlaps
                # its AP, so ordering is guaranteed, and all quadrant matmuls
                # can then accumulate in any order.
                nc.tensor.matmul(Tsc, zero1, zrhs256, start=True, stop=False,
                                 skip_group_check=True)
                vsb = []
                for k in range(CK):
                    vsb.append(big.tile([128, N], BF, tag=f"v{k}",
                                        name=f"v{k}"))
                return T0, T1, vsb

            def qk_chunk(b, hs, s):
                # qk projection for one 128-row spatial chunk
                qk = mid.tile([128, 2 * C], BF, tag="qk", bufs=4)
                for half in range(2):
                    # one-bank psum tiles (3 rotating slots) so the next
                    # chunk's matmuls never wait on this chunk's evac
                    pqk = ps_qk.tile([128, 512], F32, tag="pqk")
                    wseg = slice(half * 512, (half + 1) * 512)
                    for k in range(CK):
                        nc.tensor.matmul(
                            pqk, hs[k][:, s * 128:(s + 1) * 128],
                            wqk[k][:, wseg], start=(k == 0),
                            stop=(k == CK - 1))
                    nc.scalar.copy(out=qk[:, wseg], in_=pqk)
                # q/k bias add (bf16 tensor_tensor runs in DVE 2x mode)
                nc.vector.tensor_add(out=qk, in0=qk, in1=bqk_rep)
                return qk

            def emit_scores(qk, T0, T1):
                for h in range(NH):
                    tt, l = divmod(h, 4)
                    T = T0 if tt == 0 else T1
                    pr, cs = _SCORE_SLOT[l]
                    nc.tensor.matmul(
                        T[pr:pr + 64, cs:cs + 64],
                        qk[:, h * 64:(h + 1) * 64],
                        qk[:, 512 + h * 64:512 + (h + 1) * 64],
                        start=False, stop=False, skip_group_check=True,
                        tile_position=(0, pr))

            def phase_qkv_run(b, hs, T0, T1, vsb, s0, s1):
                # qk + scores, with the v projection interleaved (one 512-col
                # block per 4 spatial chunks) so h slices are fully consumed
                # — and released for the next batch's normalize — as the
                # loop advances.
                for s in range(s0, s1):
                    qk = qk_chunk(b, hs, s)
                    emit_scores(qk, T0, T1)
                    if s % 4 == 3:
                        t = s // 4
                        hsl = slice(t * 512, (t + 1) * 512)
                        for oc in range(CK):
                            pv = ps_big.tile([128, 512], F32, tag="pbig")
                            for k in range(CK):
                                nc.tensor.matmul(
                                    pv, wv[k][:, oc * 128:(oc + 1) * 128],
                                    hs[k][:, hsl], start=(k == 0),
                                    stop=(k == CK - 1))
                            # tensor_scalar has a 2x-mode uop (CAST is 1x)
                            nc.vector.tensor_scalar_mul(
                                out=vsb[oc][:, hsl], in0=pv, scalar1=1.0)

            def phase_att_out(b, T0, T1, vsb):
                # softmax + transpose -> attT (bf16)
                # softmax without max-subtraction: logits = S/8 are bounded
                # well inside fp32 exp range for this distribution.
                attTs = []
                for tt, T in enumerate([T0, T1]):
                    p_f = small.tile([128, 128], F32, tag=f"p{tt}")
                    att_bf = small.tile([128, 128], BF, tag=f"abf{tt}")
                    nc.scalar.activation(out=p_f, in_=T, func=AF.Exp,
                                         scale=scale)
                    rsum = small.tile([128, 2], F32, tag=f"rsum{tt}")
                    nc.vector.reduce_sum(
                        out=rsum,
                        in_=p_f.rearrange("p (h e) -> p h e", h=2),
                        axis=AX.X)
                    rinv = small.tile([128, 2], F32, tag=f"rinv{tt}")
                    nc.vector.reciprocal(out=rinv, in_=rsum)
                    for half in range(2):
                        sl = slice(half * 64, (half + 1) * 64)
                        nc.vector.tensor_scalar_mul(
                            out=att_bf[:, sl], in0=p_f[:, sl],
                            scalar1=rinv[:, half:half + 1])
                    ptr = ps_big.tile([128, 128], BF, tag="pbig")
                    nc.tensor.transpose(ptr, att_bf, ident)
                    aT = small.tile([128, 128], BF, tag=f"aT{tt}")
                    nc.vector.tensor_copy(out=aT, in_=ptr)
                    attTs.append(aT)

                # c = att @ b_v per head -> [C, 1] fp32; folded into the hv
                # evacuation as a per-partition bias (hv' = hv + c), which
                # makes w_out @ hv' carry the whole v-bias term so the output
                # only needs + b_out + x afterwards.
                csb = []
                for k in range(CK):
                    pcv = ps_big.tile([128, 1], F32, tag="pbig")
                    aT = attTs[k // 2]
                    epr, ecs = _EVEN_SLOT[k % 2]
                    opr, ocs = _ODD_SLOT[k % 2]
                    nc.tensor.matmul(
                        pcv[0:64, :], aT[epr:epr + 64, ecs:ecs + 64],
                        bv_sb[k][0:64, :], start=True, stop=True,
                        tile_position=(0, 0), skip_group_check=True)
                    nc.tensor.matmul(
                        pcv[64:128, :], aT[opr:opr + 64, ocs:ocs + 64],
                        bv_sb[k][64:128, :], start=True, stop=True,
                        tile_position=(64, 64), skip_group_check=True)
                    ct = small.tile([128, 1], F32, tag=f"c{k}")
                    nc.vector.tensor_copy(out=ct, in_=pcv)
                    csb.append(ct)

                # hv = att @ v, out = w_out @ hv + btot + x
                for t in range(NT):
                    hsl = slice(t * 512, (t + 1) * 512)
                    hvs = []
                    for k in range(CK):
                        phv = ps_big.tile([128, 512], F32, tag="pbig")
                        aT = attTs[k // 2]
                        epr, ecs = _EVEN_SLOT[k % 2]
                        opr, ocs = _ODD_SLOT[k % 2]
                        nc.tensor.matmul(
                            phv[0:64, :], aT[epr:epr + 64, ecs:ecs + 64],
                            vsb[k][0:64, hsl], start=True, stop=True,
                            tile_position=(0, 0), skip_group_check=True)
                        nc.tensor.matmul(
                            phv[64:128, :], aT[opr:opr + 64, ocs:ocs + 64],
                            vsb[k][64:128, hsl], start=True, stop=True,
                            tile_position=(64, 64), skip_group_check=True)
                        hv = mid.tile([128, 512], BF, tag=f"hv{k}", bufs=2)
                        # evacuate + add the folded v-bias (DVE 2x mode)
                        nc.vector.tensor_scalar_add(out=hv, in0=phv,
                                                    scalar1=csb[k])
                        hvs.append(hv)
                    for oc in range(CK):
                        # out-psum gets its own 2-slot tag so it never waits
                        # on hv-psum recycling (and vice versa)
                        po = ps_big.tile([128, 512], F32, tag="pout")
                        for k in range(CK):
                            nc.tensor.matmul(
                                po, wo[k][:, oc * 128:(oc + 1) * 128], hvs[k],
                                start=(k == 0), stop=(k == CK - 1))
                        xr = mid.tile([128, 512], F32, tag="xr")
                        nc.sync.dma_start(
                            out=xr,
                            in_=x_d.ap()[b, oc * 128:(oc + 1) * 128, hsl])
                        fin = mid.tile([128, 512], F32, tag="fin")
                        nc.vector.scalar_tensor_tensor(
                            out=fin, in0=po, scalar=bo_sb[oc], in1=xr,
                            op0=OP.add, op1=OP.add)
                        # non-final batches store via the idle gpsimd queue so
                        # they never delay the next batch's x loads on the
                        # sync queue; the last batch stores via sync (HWDGE)
                        # to shorten the kernel-tail drain
                        dma_eng = nc.gpsimd if b < B - 1 else nc.sync
                        dma_eng.dma_start(
                            out=out_d.ap()[b, oc * 128:(oc + 1) * 128, hsl],
                            in_=fin)

            # software-pipelined emission: batch b+1's stats/normalize AND
            # its first PRE qk-projection chunks (scores deferred to avoid
            # an in-order queue cycle) are emitted ahead of batch b's
            # softmax/hv/out, so the tensor engine has filler work while
            # batch b's softmax chain runs on DVE/ACT.
            PRE = min(3, SP)
            hs_b = phase_norm(0)
            st_b = phase_qkv_setup(0)
            phase_qkv_run(0, hs_b, *st_b, 0, SP)
            for b in range(1, B):
                hs_n = phase_norm(b)
                stash = [qk_chunk(b, hs_n, s) for s in range(PRE)]
                phase_att_out(b - 1, *st_b)
                st_b = phase_qkv_setup(b)
                for qk in stash:
                    emit_scores(qk, st_b[0], st_b[1])
                phase_qkv_run(b, hs_n, *st_b, PRE, SP)
                hs_b = hs_n
            phase_att_out(B - 1, *st_b)

    nc.compile()
    return nc


def make_indicators():
    """Host-built groupnorm reduce/broadcast indicator matrices."""
    ch = np.arange(C)
    grp = ch // (C // G)
    indf = np.zeros((C, G), np.float32)
    indf[ch, grp] = 1.0 / (C // G)
    indb = np.zeros((G, C), np.float32)
    indb[grp, ch] = 1.0
    return indf, indb


_PROGRAM = None


def _get_program():
    global _PROGRAM
    if _PROGRAM is None:
        _PROGRAM = build_program()
    return _PROGRAM


def kernel(x, gamma, beta, w_qkv, b_qkv, w_out, b_out):
    x = np.asarray(x)
    B, C_, H, W = x.shape
    N = H * W
    assert C_ == C and B == 16 and N == 4096
    nc = _get_program()

    bf = ml_dtypes.bfloat16
    w_qkv = np.asarray(w_qkv, dtype=np.float32)
    wqkT = np.ascontiguousarray(w_qkv[:2 * C].T).astype(bf)
    wvT = np.ascontiguousarray(w_qkv[2 * C:].T).astype(bf)
    woT = np.ascontiguousarray(np.asarray(w_out, dtype=np.float32).T).astype(bf)
    b_qkv = np.asarray(b_qkv, dtype=np.float32)
    bqk = np.ascontiguousarray(b_qkv[:2 * C].reshape(1, -1)).astype(bf)
    bv = np.ascontiguousarray(b_qkv[2 * C:].reshape(-1, 1)).astype(bf)
    bo = np.ascontiguousarray(np.asarray(b_out, np.float32).reshape(-1, 1))
    gam = np.ascontiguousarray(np.asarray(gamma, np.float32).reshape(-1, 1))
    bet = np.ascontiguousarray(np.asarray(beta, np.float32).reshape(-1, 1))
    xr = np.ascontiguousarray(x.reshape(B, C, N).astype(np.float32))

    indf, indb = make_indicators()
    bpc = B // N_CORES
    in_maps = []
    for c in range(N_CORES):
        in_maps.append({
            "x": xr[c * bpc:(c + 1) * bpc],
            "wqkT": wqkT, "wvT": wvT, "woT": woT,
            "bqk": bqk, "bv": bv, "bo": bo,
            "gamma": gam, "beta": bet,
            "indf": indf, "indb": indb,
        })
    res = run_bass_kernel_spmd(nc, in_maps, core_ids=list(range(N_CORES)))
    out = np.concatenate([res.results[c]["out"] for c in range(N_CORES)],
                         axis=0)
    return out.reshape(B, C_, H, W).astype(np.float32)



# revision 2
# speedup vs baseline: 1.0003x; 1.0003x over previous
"""Trainium2 Bass kernel for nn_AttentionBlock (B=16, C=512, H=W=64, 8 heads).

Channel-attention block: GroupNorm(8 groups) -> 1x1 qkv -> scores over
channel dims (contract spatial N=4096) -> softmax -> att @ v -> 1x1 out
projection -> residual.

Sharding: data-parallel over batch. 16 batches / 8 cores = 2 per core.
No collectives. Each core runs the identical program on its 2 batches.

Layouts on device (per batch):
  x     [C, N] fp32, 4 channel-chunk tiles of [128, 4096]
  h     (groupnorm output) same layout, bf16
  q,k   [N, 2C] orientation (spatial on partitions), bf16, transient tiles
  v     [C, N] bf16, resident
  scores 8 heads of [64, 64] packed into two [128, 128] psum tiles
  hv    [C, N] bf16 via paired-head matmuls
  out   = w_out @ hv + (w_out @ (att @ b_v) + b_out) + x   (residual)

All matmuls bf16 inputs with fp32 psum accumulation; groupnorm stats,
softmax, and the residual path are fp32.
"""

import numpy as np
import ml_dtypes

import concourse.bacc as bacc
import concourse.tile as tile
from concourse import mybir
from concourse.bass_utils import run_bass_kernel_spmd
from concourse.masks import make_identity

BF = mybir.dt.bfloat16
F32 = mybir.dt.float32
AX = mybir.AxisListType
OP = mybir.AluOpType
AF = mybir.ActivationFunctionType

C = 512
NH = 8
D = 64  # head dim
G = 8   # groupnorm groups
CK = C // 128  # 4 channel chunks
EPS = 1e-5
N_CORES = 8

# attT slot coords inside a [128,128] attT tile, per chunk parity.
# chunk ck holds heads (2ck, 2ck+1); tile tt = ck // 2.
# even head lhsT lives at partitions 0:64, odd head at partitions 64:128.
_EVEN_SLOT = {0: (0, 0), 1: (0, 64)}   # ck%2 -> (prow, colstart)
_ODD_SLOT = {0: (64, 64), 1: (64, 0)}
# scores placement: local head l (0..3) -> (prow, colstart) in scores tile
_SCORE_SLOT = {0: (0, 0), 1: (64, 64), 2: (64, 0), 3: (0, 64)}


def build_program(B=2, N=4096, debug=False):
    SP = N // 128   # spatial chunks for qk/scores
    NT = N // 512   # 512-col tiles
    SUB = N // 512  # bn_stats subgroups (free dim <= 512)
    scale = float(1.0 / np.sqrt(D))

    nc = bacc.Bacc("TRN2", target_bir_lowering=False, debug=debug,
                   num_devices=N_CORES)

    x_d = nc.dram_tensor("x", [B, C, N], F32, kind="ExternalInput")
    wqk_d = nc.dram_tensor("wqkT", [C, 2 * C], BF, kind="ExternalInput")
    wv_d = nc.dram_tensor("wvT", [C, C], BF, kind="ExternalInput")
    wo_d = nc.dram_tensor("woT", [C, C], BF, kind="ExternalInput")
    bqk_d = nc.dram_tensor("bqk", [1, 2 * C], BF, kind="ExternalInput")
    bv_d = nc.dram_tensor("bv", [C, 1], BF, kind="ExternalInput")
    bo_d = nc.dram_tensor("bo", [C, 1], F32, kind="ExternalInput")
    gam_d = nc.dram_tensor("gamma", [C, 1], F32, kind="ExternalInput")
    bet_d = nc.dram_tensor("beta", [C, 1], F32, kind="ExternalInput")
    indf_d = nc.dram_tensor("indf", [C, G], F32, kind="ExternalInput")
    indb_d = nc.dram_tensor("indb", [G, C], F32, kind="ExternalInput")
    out_d = nc.dram_tensor("out", [B, C, N], F32, kind="ExternalOutput")

    with tile.TileContext(nc) as tc:
        import contextlib
        ctx = contextlib.ExitStack()
        with ctx:
            persist = ctx.enter_context(tc.tile_pool(name="persist", bufs=1))
            big = ctx.enter_context(tc.tile_pool(name="big", bufs=1))
            mid = ctx.enter_context(tc.tile_pool(name="mid", bufs=3))
            small = ctx.enter_context(tc.tile_pool(name="small", bufs=1))
            ps_qk = ctx.enter_context(
                tc.tile_pool(name="ps_qk", bufs=3, space="PSUM"))
            ps_sc = ctx.enter_context(
                tc.tile_pool(name="ps_sc", bufs=1, space="PSUM"))
            ps_big = ctx.enter_context(
                tc.tile_pool(name="ps_big", bufs=2, space="PSUM"))

            # ---- persistent: weights / constants ----
            wqk = []
            wv = []
            wo = []
            bv_sb = []
            bo_sb = []
            gam = []
            bet = []
            for k in range(CK):
                t = persist.tile([128, 2 * C], BF, tag=f"wqk{k}")
                nc.gpsimd.dma_start(out=t, in_=wqk_d.ap()[k * 128:(k + 1) * 128, :])
                wqk.append(t)
                t = persist.tile([128, C], BF, tag=f"wv{k}")
                nc.gpsimd.dma_start(out=t, in_=wv_d.ap()[k * 128:(k + 1) * 128, :])
                wv.append(t)
                t = persist.tile([128, C], BF, tag=f"wo{k}")
                nc.gpsimd.dma_start(out=t, in_=wo_d.ap()[k * 128:(k + 1) * 128, :])
                wo.append(t)
                t = persist.tile([128, 1], BF, tag=f"bv{k}")
                nc.gpsimd.dma_start(out=t, in_=bv_d.ap()[k * 128:(k + 1) * 128, :])
                bv_sb.append(t)
                t = persist.tile([128, 1], F32, tag=f"bo{k}")
                nc.gpsimd.dma_start(out=t, in_=bo_d.ap()[k * 128:(k + 1) * 128, :])
                bo_sb.append(t)
                t = persist.tile([128, 1], F32, tag=f"gam{k}")
                nc.gpsimd.dma_start(out=t, in_=gam_d.ap()[k * 128:(k + 1) * 128, :])
                gam.append(t)
                t = persist.tile([128, 1], F32, tag=f"bet{k}")
                nc.gpsimd.dma_start(out=t, in_=bet_d.ap()[k * 128:(k + 1) * 128, :])
                bet.append(t)
            # q/k bias replicated across all 128 partitions (spatial rows)
            import concourse.bass as bass
            bqk_rep = persist.tile([128, 2 * C], BF, tag="bqk_rep")
            _bqk_ap = bqk_d.ap()
            nc.gpsimd.dma_start(
                out=bqk_rep,
                in_=bass.AP(tensor=_bqk_ap.tensor, offset=_bqk_ap.offset,
                            ap=[[0, 128], [1, 2 * C]]))

            zero1 = persist.tile([1, 128], BF, tag="zero1")
            nc.gpsimd.memset(zero1, 0.0)
            zrhs256 = persist.tile([1, 256], BF, tag="zrhs256")
            nc.gpsimd.memset(zrhs256, 0.0)
            ident = persist.tile([128, 128], BF, tag="ident")
            make_identity(nc, ident)
            eps_t = persist.tile([128, 1], F32, tag="eps")
            nc.gpsimd.memset(eps_t, EPS)
            # group indicator matrices (groupnorm cross-partition reduce)
            indf = []
            for k in range(CK):
                t = persist.tile([128, G], F32, tag=f"indf{k}")
                nc.gpsimd.dma_start(
                    out=t, in_=indf_d.ap()[k * 128:(k + 1) * 128, :])
                indf.append(t)
            indb = persist.tile([G, C], F32, tag="indb")
            nc.gpsimd.dma_start(out=indb, in_=indb_d.ap())

            # ---- per-batch phases (emitted software-pipelined below) ----
            def phase_norm(b):
                # x load (split DMAs so bn_stats can start on early columns)
                xs = []
                for k in range(CK):
                    t = big.tile([128, N], F32, tag=f"x{k}")
                    xq = min(1024, N)
                    for q4 in range(0, N, xq):
                        nc.sync.dma_start(
                            out=t[:, q4:q4 + xq],
                            in_=x_d.ap()[b, k * 128:(k + 1) * 128,
                                         q4:q4 + xq])
                    xs.append(t)

                # groupnorm stats: per-partition mean/var via bn_stats
                mvs = []
                for k in range(CK):
                    st = small.tile([128, SUB, 6], F32, tag=f"st{k}")
                    for j in range(SUB):
                        nc.vector.bn_stats(
                            out=st[:, j, :], in_=xs[k][:, j * 512:(j + 1) * 512])
                    mv = small.tile([128, 2], F32, tag=f"mv{k}")
                    nc.vector.bn_aggr(out=mv, in_=st)
                    mvs.append(mv)
                # rhs2: col0 = mean_p, col1 = mean_p^2 + var_p = E[x^2]_p
                rhs2s = []
                for k in range(CK):
                    r2 = small.tile([128, 2], F32, tag=f"r2{k}")
                    nc.gpsimd.tensor_copy(out=r2[:, 0:1], in_=mvs[k][:, 0:1])
                    nc.vector.scalar_tensor_tensor(
                        out=r2[:, 1:2], in0=mvs[k][:, 0:1],
                        scalar=mvs[k][:, 0:1], in1=mvs[k][:, 1:2],
                        op0=OP.mult, op1=OP.add)
                    rhs2s.append(r2)
                # cross-partition reduce to per-group stats [8, 2]
                pg = ps_big.tile([G, 2], F32, tag="pbig")
                for k in range(CK):
                    nc.tensor.matmul(pg, indf[k], rhs2s[k],
                                     start=(k == 0), stop=(k == CK - 1))
                sg = small.tile([G, 2], F32, tag="sg")
                nc.vector.tensor_copy(out=sg, in_=pg)
                t2 = small.tile([G, 1], F32, tag="t2")
                nc.vector.tensor_mul(out=t2, in0=sg[:, 0:1], in1=sg[:, 0:1])
                vs = small.tile([G, 1], F32, tag="vs")
                nc.vector.tensor_sub(out=vs, in0=sg[:, 1:2], in1=t2)
                # rstd = exp(-0.5 * ln(var + eps)); Ln/Exp share a table set
                lnv = small.tile([G, 1], F32, tag="lnv")
                nc.scalar.activation(out=lnv, in_=vs, func=AF.Ln,
                                     bias=eps_t[0:G, :], scale=1.0)
                rstd = small.tile([G, 1], F32, tag="rstd")
                nc.scalar.activation(out=rstd, in_=lnv, func=AF.Exp, scale=-0.5)
                bcr = small.tile([G, 2], F32, tag="bcr")
                nc.gpsimd.tensor_copy(out=bcr[:, 0:1], in_=sg[:, 0:1])
                nc.gpsimd.tensor_copy(out=bcr[:, 1:2], in_=rstd)
                # broadcast group stats back to channels; affine coeffs
                scs = []
                nbs = []
                for k in range(CK):
                    pbc = ps_big.tile([128, 2], F32, tag="pbig")
                    nc.tensor.matmul(pbc, indb[:, k * 128:(k + 1) * 128], bcr,
                                     start=True, stop=True)
                    sc = small.tile([128, 1], F32, tag=f"sc{k}")
                    nc.vector.tensor_mul(out=sc, in0=pbc[:, 1:2], in1=gam[k])
                    t4 = small.tile([128, 1], F32, tag=f"t4{k}")
                    nc.vector.tensor_scalar_mul(out=t4, in0=pbc[:, 0:1],
                                                scalar1=sc)
                    nb = small.tile([128, 1], F32, tag=f"nb{k}")
                    nc.vector.tensor_sub(out=nb, in0=bet[k], in1=t4)
                    scs.append(sc)
                    nbs.append(nb)

                # normalize: h = x * scale_c + bias_c  (bf16).
                # Column-major loop order: the first qk matmul needs the
                # first 128 columns of ALL FOUR chunks, so producing columns
                # across chunks first lets the consumer start ~9us earlier
                # than chunk-major order would.
                hs = []
                for k in range(CK):
                    hs.append(big.tile([128, N], BF, tag=f"h{k}",
                                       name=f"h{k}"))
                for t in range(NT):
                    sl = slice(t * 512, (t + 1) * 512)
                    for k in range(CK):
                        nc.vector.tensor_scalar(
                            out=hs[k][:, sl], in0=xs[k][:, sl],
                            scalar1=scs[k], scalar2=nbs[k],
                            op0=OP.mult, op1=OP.add)
                return hs

            def phase_qkv_setup(b):
                # scores accumulators: both packed tiles share one psum bank
                Tsc = ps_sc.tile([128, 256], F32, tag="sc01")
                T0 = Tsc[:, 0:128]
                T1 = Tsc[:, 128:256]
                # one full-width zeroing matmul: marks the bank's pending-zero
                # bits and writes 0 everywhere; every scores matmul overlaps
                # its AP, so ordering is guaranteed, and all quadrant matmuls
                # can then accumulate in any order.
                nc.tensor.matmul(Tsc, zero1, zrhs256, start=True, stop=False,
                                 skip_group_check=True)
                vsb = []
                for k in range(CK):
                    vsb.append(big.tile([128, N], BF, tag=f"v{k}",
                                        name=f"v{k}"))
                return T0, T1, vsb

            def qk_chunk(b, hs, s):
                # qk projection for one 128-row spatial chunk.
                # k-outer / half-inner: each h-slice stationary operand
                # serves two consecutive matmuls, and consecutive matmuls
                # alternate psum banks (no same-bank drain serialization).
                qk = mid.tile([128, 2 * C], BF, tag="qk", bufs=4)
                pq = ps_qk.tile([128, 512], F32, tag="pqk")
                pk = ps_qk.tile([128, 512], F32, tag="pqk")
                for k in range(CK):
                    lhs = hs[k][:, s * 128:(s + 1) * 128]
                    nc.tensor.matmul(pq, lhs, wqk[k][:, 0:512],
                                     start=(k == 0), stop=(k == CK - 1))
                    nc.tensor.matmul(pk, lhs, wqk[k][:, 512:1024],
                                     start=(k == 0), stop=(k == CK - 1))
                nc.scalar.copy(out=qk[:, 0:512], in_=pq)
                nc.scalar.copy(out=qk[:, 512:1024], in_=pk)
                # q/k bias add (bf16 tensor_tensor runs in DVE 2x mode)
                nc.vector.tensor_add(out=qk, in0=qk, in1=bqk_rep)
                return qk

            def emit_scores(qk, T0, T1):
                for h in range(NH):
                    tt, l = divmod(h, 4)
                    T = T0 if tt == 0 else T1
                    pr, cs = _SCORE_SLOT[l]
                    nc.tensor.matmul(
                        T[pr:pr + 64, cs:cs + 64],
                        qk[:, h * 64:(h + 1) * 64],
                        qk[:, 512 + h * 64:512 + (h + 1) * 64],
                        start=False, stop=False, skip_group_check=True,
                        tile_position=(0, pr))

            def phase_qkv_run(b, hs, T0, T1, vsb, s0, s1):
                # qk + scores, with the v projection interleaved (one 512-col
                # block per 4 spatial chunks) so h slices are fully consumed
                # — and released for the next batch's normalize — as the
                # loop advances.
                for s in range(s0, s1):
                    qk = qk_chunk(b, hs, s)
                    emit_scores(qk, T0, T1)
                    if s % 4 == 3:
                        t = s // 4
                        hsl = slice(t * 512, (t + 1) * 512)
                        for oc in range(CK):
                            pv = ps_big.tile([128, 512], F32, tag="pbig")
                            for k in range(CK):
                                nc.tensor.matmul(
                                    pv, wv[k][:, oc * 128:(oc + 1) * 128],
                                    hs[k][:, hsl], start=(k == 0),
                                    stop=(k == CK - 1))
                            # tensor_scalar has a 2x-mode uop (CAST is 1x)
                            nc.vector.tensor_scalar_mul(
                                out=vsb[oc][:, hsl], in0=pv, scalar1=1.0)

            def phase_att_out(b, T0, T1, vsb):
                # softmax + transpose -> attT (bf16)
                # softmax without max-subtraction: logits = S/8 are bounded
                # well inside fp32 exp range for this distribution.
                attTs = []
                for tt, T in enumerate([T0, T1]):
                    p_f = small.tile([128, 128], F32, tag=f"p{tt}")
                    att_bf = small.tile([128, 128], BF, tag=f"abf{tt}")
                    nc.scalar.activation(out=p_f, in_=T, func=AF.Exp,
                                         scale=scale)
                    rsum = small.tile([128, 2], F32, tag=f"rsum{tt}")
                    nc.vector.reduce_sum(
                        out=rsum,
                        in_=p_f.rearrange("p (h e) -> p h e", h=2),
                        axis=AX.X)
                    rinv = small.tile([128, 2], F32, tag=f"rinv{tt}")
                    nc.vector.reciprocal(out=rinv, in_=rsum)
                    for half in range(2):
                        sl = slice(half * 64, (half + 1) * 64)
                        nc.vector.tensor_scalar_mul(
                            out=att_bf[:, sl], in0=p_f[:, sl],
                            scalar1=rinv[:, half:half + 1])
                    ptr = ps_big.tile([128, 128], BF, tag="pbig")
                    nc.tensor.transpose(ptr, att_bf, ident)
                    aT = small.tile([128, 128], BF, tag=f"aT{tt}")
                    nc.vector.tensor_copy(out=aT, in_=ptr)
                    attTs.append(aT)

                # c = att @ b_v per head -> [C, 1] fp32; folded into the hv
                # evacuation as a per-partition bias (hv' = hv + c), which
                # makes w_out @ hv' carry the whole v-bias term so the output
                # only needs + b_out + x afterwards.
                csb = []
                for k in range(CK):
                    pcv = ps_big.tile([128, 1], F32, tag="pbig")
                    aT = attTs[k // 2]
                    epr, ecs = _EVEN_SLOT[k % 2]
                    opr, ocs = _ODD_SLOT[k % 2]
                    nc.tensor.matmul(
                        pcv[0:64, :], aT[epr:epr + 64, ecs:ecs + 64],
                        bv_sb[k][0:64, :], start=True, stop=True,
                        tile_position=(0, 0), skip_group_check=True)
                    nc.tensor.matmul(
                        pcv[64:128, :], aT[opr:opr + 64, ocs:ocs + 64],
                        bv_sb[k][64:128, :], start=True, stop=True,
                        tile_position=(64, 64), skip_group_check=True)
                    ct = small.tile([128, 1], F32, tag=f"c{k}")
                    nc.vector.tensor_copy(out=ct, in_=pcv)
                    csb.append(ct)

                # hv = att @ v, out = w_out @ hv + btot + x
                for t in range(NT):
                    hsl = slice(t * 512, (t + 1) * 512)
                    hvs = []
                    for k in range(CK):
                        phv = ps_big.tile([128, 512], F32, tag="pbig")
                        aT = attTs[k // 2]
                        epr, ecs = _EVEN_SLOT[k % 2]
                        opr, ocs = _ODD_SLOT[k % 2]
                        nc.tensor.matmul(
                            phv[0:64, :], aT[epr:epr + 64, ecs:ecs + 64],
                            vsb[k][0:64, hsl], start=True, stop=True,
                            tile_position=(0, 0), skip_group_check=True)
                        nc.tensor.matmul(
                            phv[64:128, :], aT[opr:opr + 64, ocs:ocs + 64],
                            vsb[k][64:128, hsl], start=True, stop=True,
                            tile_position=(64, 64), skip_group_check=True)
                        hv = mid.tile([128, 512], BF, tag=f"hv{k}", bufs=2)
                        # evacuate + add the folded v-bias (DVE 2x mode)
                        nc.vector.tensor_scalar_add(out=hv, in0=phv,
                                                    scalar1=csb[k])
                        hvs.append(hv)
                    for oc in range(CK):
                        # out-psum gets its own 2-slot tag so it never waits
                        # on hv-psum recycling (and vice versa)
                        po = ps_big.tile([128, 512], F32, tag="pout")
                        for k in range(CK):
                            nc.tensor.matmul(
                                po, wo[k][:, oc * 128:(oc + 1) * 128], hvs[k],
                                start=(k == 0), stop=(k == CK - 1))
                        xr = mid.tile([128, 512], F32, tag="xr")
                        nc.sync.dma_start(
                            out=xr,
                            in_=x_d.ap()[b, oc * 128:(oc + 1) * 128, hsl])
                        fin = mid.tile([128, 512], F32, tag="fin")
                        nc.vector.scalar_tensor_tensor(
                            out=fin, in0=po, scalar=bo_sb[oc], in1=xr,
                            op0=OP.add, op1=OP.add)
                        # non-final batches store via the idle gpsimd queue so
                        # they never delay the next batch's x loads on the
                        # sync queue; the last batch stores via sync (HWDGE)
                        # to shorten the kernel-tail drain
                        dma_eng = nc.gpsimd if b < B - 1 else nc.sync
                        dma_eng.dma_start(
                            out=out_d.ap()[b, oc * 128:(oc + 1) * 128, hsl],
                            in_=fin)

            # software-pipelined emission: batch b+1's stats/normalize AND
            # its first PRE qk-projection chunks (scores deferred to avoid
            # an in-order queue cycle) are emitted ahead of batch b's
            # softmax/hv/out, so the tensor engine has filler work while
            # batch b's softmax chain runs on DVE/ACT.
            PRE = min(3, SP)
            hs_b = phase_norm(0)
            st_b = phase_qkv_setup(0)
            phase_qkv_run(0, hs_b, *st_b, 0, SP)
            for b in range(1, B):
                hs_n = phase_norm(b)
                stash = [qk_chunk(b, hs_n, s) for s in range(PRE)]
                phase_att_out(b - 1, *st_b)
                st_b = phase_qkv_setup(b)
                for qk in stash:
                    emit_scores(qk, st_b[0], st_b[1])
                phase_qkv_run(b, hs_n, *st_b, PRE, SP)
                hs_b = hs_n
            phase_att_out(B - 1, *st_b)

    nc.compile()
    return nc


def make_indicators():
    """Host-built groupnorm reduce/broadcast indicator matrices."""
    ch = np.arange(C)
    grp = ch // (C // G)
    indf = np.zeros((C, G), np.float32)
    indf[ch, grp] = 1.0 / (C // G)
    indb = np.zeros((G, C), np.float32)
    indb[grp, ch] = 1.0
    return indf, indb


_PROGRAM = None


def _get_program():
    global _PROGRAM
    if _PROGRAM is None:
        _PROGRAM = build_program()
    return _PROGRAM


def kernel(x, gamma, beta, w_qkv, b_qkv, w_out, b_out):
    x = np.asarray(x)
    B, C_, H, W = x.shape
    N = H * W
    assert C_ == C and B == 16 and N == 4096
    nc = _get_program()

    bf = ml_dtypes.bfloat16
    w_qkv = np.asarray(w_qkv, dtype=np.float32)
    wqkT = np.ascontiguousarray(w_qkv[:2 * C].T).astype(bf)
    wvT = np.ascontiguousarray(w_qkv[2 * C:].T).astype(bf)
    woT = np.ascontiguousarray(np.asarray(w_out, dtype=np.float32).T).astype(bf)
    b_qkv = np.asarray(b_qkv, dtype=np.float32)
    bqk = np.ascontiguousarray(b_qkv[:2 * C].reshape(1, -1)).astype(bf)
    bv = np.ascontiguousarray(b_qkv[2 * C:].reshape(-1, 1)).astype(bf)
    bo = np.ascontiguousarray(np.asarray(b_out, np.float32).reshape(-1, 1))
    gam = np.ascontiguousarray(np.asarray(gamma, np.float32).reshape(-1, 1))
    bet = np.ascontiguousarray(np.asarray(beta, np.float32).reshape(-1, 1))
    xr = np.ascontiguousarray(x.reshape(B, C, N).astype(np.float32))

    indf, indb = make_indicators()
    bpc = B // N_CORES
    in_maps = []
    for c in range(N_CORES):
        in_maps.append({
            "x": xr[c * bpc:(c + 1) * bpc],
            "wqkT": wqkT, "wvT": wvT, "woT": woT,
            "bqk": bqk, "bv": bv, "bo": bo,
            "gamma": gam, "beta": bet,
            "indf": indf, "indb": indb,
        })
    res = run_bass_kernel_spmd(nc, in_maps, core_ids=list(range(N_CORES)))
    out = np.concatenate([res.results[c]["out"] for c in range(N_CORES)],
                         axis=0)
    return out.reshape(B, C_, H, W).astype(np.float32)



# revision 3
# speedup vs baseline: 1.3166x; 1.3162x over previous
"""Trainium2 Bass kernel for nn_AttentionBlock (B=16, C=512, H=W=64, 8 heads).

Channel-attention block: GroupNorm(8 groups) -> 1x1 qkv -> scores over
channel dims (contract spatial N=4096) -> softmax -> att @ v -> 1x1 out
projection -> residual.

Sharding: data-parallel over batch. 16 batches / 8 cores = 2 per core.
No collectives. Each core runs the identical program on its 2 batches.

Key structure (v2):
  x arrives bf16 (host-converted): halves load DMA, lets bn_stats run in
    DVE 2x mode and normalize in 4x mode; x stays resident (bufs=2) for
    the residual add, so there is no second x load.
  v bias is folded into the v-psum evacuation (ACT Identity+bias), so
    hv = att @ (wv h + bv) needs no separate correction.
  out projection runs in fp8 DoubleRow (wo and hv quantized to e4m3,
    contraction 256 per matmul) - halves the out-proj matmul count.
  Software pipeline: batch b+1's bn_stats + groupnorm + normalize are
    interleaved into batch b's qk phase (where DVE has slack), so the
    attention window of batch b only carries softmax + fin on DVE and
    the batch transition has PE work immediately available.
  Scores matmuls are deferred one chunk behind the qk projection, and
    two v-blocks are deferred to the attention window, so the PE never
    waits on the qk-evac/bias or softmax chains.
"""

import numpy as np
import ml_dtypes

import concourse.bacc as bacc
import concourse.tile as tile
from concourse import mybir
from concourse.bass_utils import run_bass_kernel_spmd
from concourse.masks import make_identity

BF = mybir.dt.bfloat16
F32 = mybir.dt.float32
F8 = mybir.dt.float8e4
AX = mybir.AxisListType
OP = mybir.AluOpType
AF = mybir.ActivationFunctionType
DR = mybir.MatmulPerfMode.DoubleRow

C = 512
NH = 8
D = 64  # head dim
G = 8   # groupnorm groups
CK = C // 128  # 4 channel chunks
EPS = 1e-5
N_CORES = 8

# attT slot coords inside a [128,128] attT tile, per chunk parity.
# chunk ck holds heads (2ck, 2ck+1); tile tt = ck // 2.
_EVEN_SLOT = {0: (0, 0), 1: (0, 64)}   # ck%2 -> (prow, colstart)
_ODD_SLOT = {0: (64, 64), 1: (64, 0)}
# scores placement: local head l (0..3) -> (prow, colstart) in scores tile
_SCORE_SLOT = {0: (0, 0), 1: (64, 64), 2: (64, 0), 3: (0, 64)}


def build_program(B=2, N=4096, debug=False, use_fp8_out=True):
    SP = N // 128   # spatial chunks for qk/scores
    NT = N // 512   # 512-col tiles
    SUB = N // 512  # bn_stats subgroups (free dim <= 512)
    scale = float(1.0 / np.sqrt(D))

    nc = bacc.Bacc("TRN2", target_bir_lowering=False, debug=debug,
                   num_devices=N_CORES)

    x_d = nc.dram_tensor("x", [B, C, N], BF, kind="ExternalInput")
    wqk_d = nc.dram_tensor("wqkT", [C, 2 * C], BF, kind="ExternalInput")
    wv_d = nc.dram_tensor("wvT", [C, C], BF, kind="ExternalInput")
    if use_fp8_out:
        # [kpair, p, i, o]: contraction chunk c = kpair*256 + i*128 + p
        wo_d = nc.dram_tensor("wo8", [2, 128, 2, C], F8, kind="ExternalInput")
    else:
        wo_d = nc.dram_tensor("woT", [C, C], BF, kind="ExternalInput")
    bqk_d = nc.dram_tensor("bqk", [1, 2 * C], BF, kind="ExternalInput")
    bv_d = nc.dram_tensor("bv", [C, 1], F32, kind="ExternalInput")
    bo_d = nc.dram_tensor("bo", [C, 1], F32, kind="ExternalInput")
    gam_d = nc.dram_tensor("gamma", [C, 1], F32, kind="ExternalInput")
    bet_d = nc.dram_tensor("beta", [C, 1], F32, kind="ExternalInput")
    indf_d = nc.dram_tensor("indf", [C, G], F32, kind="ExternalInput")
    indb_d = nc.dram_tensor("indb", [G, C], F32, kind="ExternalInput")
    out_d = nc.dram_tensor("out", [B, C, N], F32, kind="ExternalOutput")

    with tile.TileContext(nc) as tc:
        import contextlib
        import concourse.bass as bass
        ctx = contextlib.ExitStack()
        with ctx:
            persist = ctx.enter_context(tc.tile_pool(name="persist", bufs=1))
            big = ctx.enter_context(tc.tile_pool(name="big", bufs=1))
            mid = ctx.enter_context(tc.tile_pool(name="mid", bufs=3))
            small = ctx.enter_context(tc.tile_pool(name="small", bufs=1))
            ps_qk = ctx.enter_context(
                tc.tile_pool(name="ps_qk", bufs=3, space="PSUM"))
            ps_sc = ctx.enter_context(
                tc.tile_pool(name="ps_sc", bufs=1, space="PSUM"))
            ps_big = ctx.enter_context(
                tc.tile_pool(name="ps_big", bufs=2, space="PSUM"))

            # ---- persistent: weights / constants ----
            wqk = []
            wv = []
            wo = []
            bv_sb = []
            bo_sb = []
            gam = []
            bet = []
            for k in range(CK):
                t = persist.tile([128, 2 * C], BF, tag=f"wqk{k}")
                nc.gpsimd.dma_start(out=t, in_=wqk_d.ap()[k * 128:(k + 1) * 128, :])
                wqk.append(t)
                t = persist.tile([128, C], BF, tag=f"wv{k}")
                nc.gpsimd.dma_start(out=t, in_=wv_d.ap()[k * 128:(k + 1) * 128, :])
                wv.append(t)
                t = persist.tile([128, 1], F32, tag=f"bv{k}")
                nc.gpsimd.dma_start(out=t, in_=bv_d.ap()[k * 128:(k + 1) * 128, :])
                bv_sb.append(t)
                t = persist.tile([128, 1], F32, tag=f"bo{k}")
                nc.gpsimd.dma_start(out=t, in_=bo_d.ap()[k * 128:(k + 1) * 128, :])
                bo_sb.append(t)
                t = persist.tile([128, 1], F32, tag=f"gam{k}")
                nc.gpsimd.dma_start(out=t, in_=gam_d.ap()[k * 128:(k + 1) * 128, :])
                gam.append(t)
                t = persist.tile([128, 1], F32, tag=f"bet{k}")
                nc.gpsimd.dma_start(out=t, in_=bet_d.ap()[k * 128:(k + 1) * 128, :])
                bet.append(t)
            if use_fp8_out:
                for kp in range(2):
                    t = persist.tile([128, 2, C], F8, tag=f"wo8_{kp}")
                    nc.gpsimd.dma_start(out=t, in_=wo_d.ap()[kp])
                    wo.append(t)
            else:
                for k in range(CK):
                    t = persist.tile([128, C], BF, tag=f"wo{k}")
                    nc.gpsimd.dma_start(
                        out=t, in_=wo_d.ap()[k * 128:(k + 1) * 128, :])
                    wo.append(t)
            # q/k bias replicated across all 128 partitions (spatial rows)
            bqk_rep = persist.tile([128, 2 * C], BF, tag="bqk_rep")
            _bqk_ap = bqk_d.ap()
            nc.gpsimd.dma_start(
                out=bqk_rep,
                in_=bass.AP(tensor=_bqk_ap.tensor, offset=_bqk_ap.offset,
                            ap=[[0, 128], [1, 2 * C]]))

            zero1 = persist.tile([1, 128], BF, tag="zero1")
            nc.gpsimd.memset(zero1, 0.0)
            zrhs256 = persist.tile([1, 256], BF, tag="zrhs256")
            nc.gpsimd.memset(zrhs256, 0.0)
            ident = persist.tile([128, 128], BF, tag="ident")
            make_identity(nc, ident)
            eps_t = persist.tile([128, 1], F32, tag="eps")
            nc.gpsimd.memset(eps_t, EPS)
            indf = []
            for k in range(CK):
                t = persist.tile([128, G], F32, tag=f"indf{k}")
                nc.gpsimd.dma_start(
                    out=t, in_=indf_d.ap()[k * 128:(k + 1) * 128, :])
                indf.append(t)
            indb = persist.tile([G, C], F32, tag="indb")
            nc.gpsimd.dma_start(out=indb, in_=indb_d.ap())

            # ---- per-batch state ----
            state = {}  # b -> dict with xs, hs, st, mv, scs, nbs, ...

            def load_x(b):
                st = state.setdefault(b, {})
                xs = []
                for k in range(CK):
                    t = big.tile([128, N], BF, tag=f"x{k}", bufs=2,
                                 name=f"x{k}")
                    for q4 in range(0, N, 1024):
                        nc.sync.dma_start(
                            out=t[:, q4:q4 + 1024],
                            in_=x_d.ap()[b, k * 128:(k + 1) * 128,
                                         q4:q4 + 1024])
                    xs.append(t)
                st["xs"] = xs

            def stats_part(b, j):
                # bn_stats for 512-col slice j, all 4 channel chunks
                st = state[b]
                if "st" not in st:
                    st["st"] = [small.tile([128, SUB, 6], F32, tag=f"st{k}",
                                           name=f"st{k}")
                                for k in range(CK)]
                for k in range(CK):
                    nc.vector.bn_stats(
                        out=st["st"][k][:, j, :],
                        in_=st["xs"][k][:, j * 512:(j + 1) * 512])

            def gn_reduce(b):
                # aggregate stats and produce per-channel scale/bias
                stt = state[b]
                mvs = []
                for k in range(CK):
                    mv = small.tile([128, 2], F32, tag=f"mv{k}", name=f"mv{k}")
                    nc.vector.bn_aggr(out=mv, in_=stt["st"][k])
                    mvs.append(mv)
                rhs2s = []
                for k in range(CK):
                    r2 = small.tile([128, 2], F32, tag=f"r2{k}", name=f"r2{k}")
                    nc.gpsimd.tensor_copy(out=r2[:, 0:1], in_=mvs[k][:, 0:1])
                    nc.vector.scalar_tensor_tensor(
                        out=r2[:, 1:2], in0=mvs[k][:, 0:1],
                        scalar=mvs[k][:, 0:1], in1=mvs[k][:, 1:2],
                        op0=OP.mult, op1=OP.add)
                    rhs2s.append(r2)
                pg = ps_big.tile([G, 2], F32, tag="pout", name="pg")
                for k in range(CK):
                    nc.tensor.matmul(pg, indf[k], rhs2s[k],
                                     start=(k == 0), stop=(k == CK - 1))
                sg = small.tile([G, 2], F32, tag="sg", name="sg")
                nc.vector.tensor_copy(out=sg, in_=pg)
                t2 = small.tile([G, 1], F32, tag="t2", name="t2")
                nc.vector.tensor_mul(out=t2, in0=sg[:, 0:1], in1=sg[:, 0:1])
                vs = small.tile([G, 1], F32, tag="vs", name="vs")
                nc.vector.tensor_sub(out=vs, in0=sg[:, 1:2], in1=t2)
                lnv = small.tile([G, 1], F32, tag="lnv", name="lnv")
                nc.scalar.activation(out=lnv, in_=vs, func=AF.Ln,
                                     bias=eps_t[0:G, :], scale=1.0)
                rstd = small.tile([G, 1], F32, tag="rstd", name="rstd")
                nc.scalar.activation(out=rstd, in_=lnv, func=AF.Exp, scale=-0.5)
                bcr = small.tile([G, 2], F32, tag="bcr", name="bcr")
                nc.gpsimd.tensor_copy(out=bcr[:, 0:1], in_=sg[:, 0:1])
                nc.gpsimd.tensor_copy(out=bcr[:, 1:2], in_=rstd)
                scs = []
                nbs = []
                for k in range(CK):
                    pbc = ps_big.tile([128, 2], F32, tag="pout", name="pbc")
                    nc.tensor.matmul(pbc, indb[:, k * 128:(k + 1) * 128], bcr,
                                     start=True, stop=True)
                    sc = small.tile([128, 1], F32, tag=f"sc{k}", name=f"sc{k}")
                    nc.vector.tensor_mul(out=sc, in0=pbc[:, 1:2], in1=gam[k])
                    t4 = small.tile([128, 1], F32, tag=f"t4{k}", name=f"t4{k}")
                    nc.vector.tensor_scalar_mul(out=t4, in0=pbc[:, 0:1],
                                                scalar1=sc)
                    nb = small.tile([128, 1], F32, tag=f"nb{k}", name=f"nb{k}")
                    nc.vector.tensor_sub(out=nb, in0=bet[k], in1=t4)
                    scs.append(sc)
                    nbs.append(nb)
                stt["scs"] = scs
                stt["nbs"] = nbs
                stt["hs"] = [big.tile([128, N], BF, tag=f"h{k}", bufs=2,
                                      name=f"h{k}") for k in range(CK)]

            def norm_part(b, j):
                # normalize 512-col slice j (bf16 in/out -> DVE 4x mode)
                stt = state[b]
                sl = slice(j * 512, (j + 1) * 512)
                for k in range(CK):
                    nc.vector.tensor_scalar(
                        out=stt["hs"][k][:, sl], in0=stt["xs"][k][:, sl],
                        scalar1=stt["scs"][k], scalar2=stt["nbs"][k],
                        op0=OP.mult, op1=OP.add)

            def setup_scores(b):
                stt = state[b]
                Tsc = ps_sc.tile([128, 256], F32, tag="sc01", name="Tsc")
                nc.tensor.matmul(Tsc, zero1, zrhs256, start=True, stop=False,
                                 skip_group_check=True)
                stt["Tsc"] = Tsc
                stt["vsb"] = [big.tile([128, N], BF, tag=f"v{k}",
                                       name=f"v{k}") for k in range(CK)]

            def qk_chunk(b, s):
                stt = state[b]
                hs = stt["hs"]
                qk = mid.tile([128, 2 * C], BF, tag="qk", bufs=4, name="qk")
                pq = ps_qk.tile([128, 512], F32, tag="pqk", name="pq")
                pk = ps_qk.tile([128, 512], F32, tag="pqk", name="pk")
                for k in range(CK):
                    nc.tensor.matmul(pq, hs[k][:, s * 128:(s + 1) * 128],
                                     wqk[k][:, 0:512], start=(k == 0),
                                     stop=(k == CK - 1))
                for k in range(CK):
                    nc.tensor.matmul(pk, hs[k][:, s * 128:(s + 1) * 128],
                                     wqk[k][:, 512:1024], start=(k == 0),
                                     stop=(k == CK - 1))
                nc.scalar.copy(out=qk[:, 0:512], in_=pq)
                nc.vector.tensor_copy(out=qk[:, 512:1024], in_=pk)
                nc.vector.tensor_add(out=qk, in0=qk, in1=bqk_rep)
                return qk

            def emit_scores(b, qk):
                T = state[b]["Tsc"]
                T0 = T[:, 0:128]
                T1 = T[:, 128:256]
                for h in range(NH):
                    tt, l = divmod(h, 4)
                    Tt = T0 if tt == 0 else T1
                    pr, cs = _SCORE_SLOT[l]
                    nc.tensor.matmul(
                        Tt[pr:pr + 64, cs:cs + 64],
                        qk[:, h * 64:(h + 1) * 64],
                        qk[:, 512 + h * 64:512 + (h + 1) * 64],
                        start=False, stop=False, skip_group_check=True,
                        tile_position=(0, pr))

            def v_block(b, t):
                stt = state[b]
                hsl = slice(t * 512, (t + 1) * 512)
                for oc in range(CK):
                    pv = ps_big.tile([128, 512], F32, tag="pbig", name="pv")
                    for k in range(CK):
                        nc.tensor.matmul(
                            pv, wv[k][:, oc * 128:(oc + 1) * 128],
                            stt["hs"][k][:, hsl], start=(k == 0),
                            stop=(k == CK - 1))
                    # v-bias folded into the evacuation: v = wv h + bv
                    nc.scalar.add(out=stt["vsb"][oc][:, hsl], in_=pv,
                                  add=bv_sb[oc])

            def softmax(b):
                stt = state[b]
                T = stt["Tsc"]
                attTs = []
                for tt in range(2):
                    Tt = T[:, tt * 128:(tt + 1) * 128]
                    p_f = small.tile([128, 128], F32, tag=f"p{tt}",
                                     name=f"p{tt}")
                    att_bf = small.tile([128, 128], BF, tag=f"abf{tt}",
                                        name=f"abf{tt}")
                    nc.scalar.activation(out=p_f, in_=Tt, func=AF.Exp,
                                         scale=scale)
                    rsum = small.tile([128, 2], F32, tag=f"rsum{tt}",
                                      name=f"rsum{tt}")
                    nc.vector.reduce_sum(
                        out=rsum,
                        in_=p_f.rearrange("p (h e) -> p h e", h=2),
                        axis=AX.X)
                    rinv = small.tile([128, 2], F32, tag=f"rinv{tt}",
                                      name=f"rinv{tt}")
                    nc.vector.reciprocal(out=rinv, in_=rsum)
                    for half in range(2):
                        sl = slice(half * 64, (half + 1) * 64)
                        nc.vector.tensor_scalar_mul(
                            out=att_bf[:, sl], in0=p_f[:, sl],
                            scalar1=rinv[:, half:half + 1])
                    ptr = ps_big.tile([128, 128], BF, tag="pbig", name="ptr")
                    nc.tensor.transpose(ptr, att_bf, ident)
                    aT = small.tile([128, 128], BF, tag=f"aT{tt}",
                                    name=f"aT{tt}")
                    nc.vector.tensor_copy(out=aT, in_=ptr)
                    attTs.append(aT)
                stt["attTs"] = attTs

            def hv_t(b, t):
                stt = state[b]
                hsl = slice(t * 512, (t + 1) * 512)
                hv2s = []
                for kp in range(2):
                    hv2 = mid.tile([128, 1024], F8 if use_fp8_out else BF,
                                   tag="hv2", bufs=2, name="hv2")
                    for j in range(2):
                        k = kp * 2 + j
                        aT = stt["attTs"][k // 2]
                        epr, ecs = _EVEN_SLOT[k % 2]
                        opr, ocs = _ODD_SLOT[k % 2]
                        phv = ps_big.tile([128, 512], F32, tag="pbig",
                                          name="phv")
                        nc.tensor.matmul(
                            phv[0:64, :], aT[epr:epr + 64, ecs:ecs + 64],
                            stt["vsb"][k][0:64, hsl], start=True, stop=True,
                            tile_position=(0, 0), skip_group_check=True)
                        nc.tensor.matmul(
                            phv[64:128, :], aT[opr:opr + 64, ocs:ocs + 64],
                            stt["vsb"][k][64:128, hsl], start=True, stop=True,
                            tile_position=(64, 64), skip_group_check=True)
                        nc.scalar.copy(out=hv2[:, j * 512:(j + 1) * 512],
                                       in_=phv)
                    hv2s.append(hv2)
                return hv2s

            def out_t(b, t, hv2s, last_batch):
                stt = state[b]
                hsl = slice(t * 512, (t + 1) * 512)
                for oc in range(CK):
                    po = ps_big.tile([128, 512], F32, tag="pout", name="po")
                    if use_fp8_out:
                        for kp in range(2):
                            nc.tensor.matmul(
                                po,
                                wo[kp][:, :, oc * 128:(oc + 1) * 128],
                                hv2s[kp].rearrange("p (i n) -> p i n", i=2),
                                start=(kp == 0), stop=(kp == 1),
                                perf_mode=DR)
                    else:
                        for kp in range(2):
                            for j in range(2):
                                k = kp * 2 + j
                                nc.tensor.matmul(
                                    po, wo[k][:, oc * 128:(oc + 1) * 128],
                                    hv2s[kp][:, j * 512:(j + 1) * 512],
                                    start=(k == 0), stop=(k == CK - 1))
                    fin = mid.tile([128, 512], F32, tag="fin", bufs=2,
                                   name="fin")
                    nc.vector.scalar_tensor_tensor(
                        out=fin, in0=po, scalar=bo_sb[oc],
                        in1=stt["xs"][oc][:, hsl], op0=OP.add, op1=OP.add)
                    dma_eng = nc.sync if last_batch else nc.gpsimd
                    dma_eng.dma_start(
                        out=out_d.ap()[b, oc * 128:(oc + 1) * 128, hsl],
                        in_=fin)

            # ================= emission =================
            # prologue: batch 0 norm standalone
            load_x(0)
            for j in range(SUB):
                stats_part(0, j)
            gn_reduce(0)
            for j in range(NT):
                norm_part(0, j)
            setup_scores(0)

            for b in range(B):
                nxt = b + 1 if b + 1 < B else None
                if nxt is not None:
                    load_x(nxt)
                # ---- qk phase for b, with b+1 norm interleaved ----
                prev_qk = None
                for s in range(SP):
                    qk = qk_chunk(b, s)
                    if prev_qk is not None:
                        emit_scores(b, prev_qk)
                    prev_qk = qk
                    if s % 4 == 3 and s < 24:
                        v_block(b, s // 4)
                    if nxt is not None:
                        if 10 <= s < 10 + SUB:
                            stats_part(nxt, s - 10)
                        elif s == 10 + SUB + 1:
                            gn_reduce(nxt)
                        elif 20 <= s < 20 + NT:
                            norm_part(nxt, s - 20)
                emit_scores(b, prev_qk)
                # ---- attention window for b ----
                v_block(b, 6)
                v_block(b, 7)
                softmax(b)
                if nxt is not None:
                    setup_scores(nxt)
                for t in range(NT):
                    hv2s = hv_t(b, t)
                    out_t(b, t, hv2s, last_batch=(nxt is None))
                state.pop(b - 1, None)

    nc.compile()
    return nc


def make_indicators():
    """Host-built groupnorm reduce/broadcast indicator matrices."""
    ch = np.arange(C)
    grp = ch // (C // G)
    indf = np.zeros((C, G), np.float32)
    indf[ch, grp] = 1.0 / (C // G)
    indb = np.zeros((G, C), np.float32)
    indb[grp, ch] = 1.0
    return indf, indb


def prep_inputs(x, gamma, beta, w_qkv, b_qkv, w_out, b_out, use_fp8_out=True):
    """Host-side input prep shared by kernel() and test harness."""
    bf = ml_dtypes.bfloat16
    f8 = ml_dtypes.float8_e4m3fn
    B, C_, H, W = x.shape
    N = H * W
    w_qkv = np.asarray(w_qkv, dtype=np.float32)
    wqkT = np.ascontiguousarray(w_qkv[:2 * C].T).astype(bf)
    wvT = np.ascontiguousarray(w_qkv[2 * C:].T).astype(bf)
    woT = np.ascontiguousarray(np.asarray(w_out, dtype=np.float32).T)
    b_qkv = np.asarray(b_qkv, dtype=np.float32)
    bqk = np.ascontiguousarray(b_qkv[:2 * C].reshape(1, -1)).astype(bf)
    bv = np.ascontiguousarray(b_qkv[2 * C:].reshape(-1, 1).astype(np.float32))
    bo = np.ascontiguousarray(np.asarray(b_out, np.float32).reshape(-1, 1))
    gam = np.ascontiguousarray(np.asarray(gamma, np.float32).reshape(-1, 1))
    bet = np.ascontiguousarray(np.asarray(beta, np.float32).reshape(-1, 1))
    xr = np.ascontiguousarray(
        np.asarray(x, np.float32).reshape(B, C, N)).astype(bf)
    indf, indb = make_indicators()
    base = {
        "wqkT": wqkT, "wvT": wvT,
        "bqk": bqk, "bv": bv, "bo": bo,
        "gamma": gam, "beta": bet,
        "indf": indf, "indb": indb,
    }
    if use_fp8_out:
        # [kpair, p, i, o]: c = kpair*256 + i*128 + p
        wo8 = np.ascontiguousarray(
            woT.reshape(2, 2, 128, C).transpose(0, 2, 1, 3)).astype(f8)
        base["wo8"] = wo8
    else:
        base["woT"] = woT.astype(bf)
    return xr, base


_PROGRAM = None


def _get_program():
    global _PROGRAM
    if _PROGRAM is None:
        _PROGRAM = build_program()
    return _PROGRAM


def kernel(x, gamma, beta, w_qkv, b_qkv, w_out, b_out):
    x = np.asarray(x)
    B, C_, H, W = x.shape
    N = H * W
    assert C_ == C and B == 16 and N == 4096
    nc = _get_program()
    xr, base = prep_inputs(x, gamma, beta, w_qkv, b_qkv, w_out, b_out)
    bpc = B // N_CORES
    in_maps = []
    for c in range(N_CORES):
        m = dict(base)
        m["x"] = xr[c * bpc:(c + 1) * bpc]
        in_maps.append(m)
    res = run_bass_kernel_spmd(nc, in_maps, core_ids=list(range(N_CORES)))
    out = np.concatenate([res.results[c]["out"] for c in range(N_CORES)],
                         axis=0)
    return out.reshape(B, C_, H, W).astype(np.float32)
